# revision 1
# baseline (speedup 1.0000x reference)
"""MoE audio projector kernel for 8 Trainium2 NeuronCores.

Strategy (expert-parallel, sparse dispatch):
  Host: depthwise conv + residual, fold K frames, RMSNorm, sigmoid router,
        top-2 + combine weights, per-expert token gather (all tiny FLOPs).
  Device (8 cores): core c handles expert c//2 with H-half c%2 over only the
        tokens routed to that expert, plus a 1/8 H-slice of the shared
        expert over all tokens. bf16 matmuls, fp32 PSUM accumulation.
  Host: sum shared partials, scatter-add combine-scaled expert partials.

DMA design: every dma_start has a fixed ~625ns issue cost on the shared
HWDGE path, so inputs are pre-tiled on the host into layouts that give few,
large, per-partition-contiguous transfers (~42 DMAs/core instead of ~550).
"""

import math

import numpy as np
import ml_dtypes

import concourse.bass as bass
import concourse.bacc as bacc
import concourse.mybir as mybir
import concourse.tile as tile
from concourse.bass_utils import run_bass_kernel_spmd

BF16 = ml_dtypes.bfloat16
P = 128
B, S, D = 4, 1024, 1280
KF = 4                  # frames folded per token
IN = D * KF             # 5120
H = 2048
O = 2048
E = 4
TOPK = 2
TK = B * (S // KF)      # 1024 tokens
KT = IN // P            # 40 contraction tiles
KG = 8                  # k-tiles per DMA group
NKG = KT // KG          # 5 groups
H1E = H // 2            # expert H half per core
ME = H1E // P           # 8
H1S = H // 8            # shared H slice per core
MS = H1S // P           # 2
NO = O // 512           # 4 output col tiles
EPS_RMS = 1e-8
EPS_W = 1e-6
NCORES = 8


def _chunks(total, step):
    """Split `total` into ceil(total/step) near-equal chunks (each <= step)."""
    n = (total + step - 1) // step
    base = total // n
    rem = total - base * n
    out = []
    off = 0
    for i in range(n):
        w = base + (1 if i < rem else 0)
        out.append((off, w))
        off += w
    return out


def host_preprocess(x, conv_w, conv_b, rms_w, router_w):
    """conv + fold + rmsnorm + router; returns (n [TK, IN] f32, combine [TK, E] f32)."""
    xp = np.pad(x, ((0, 0), (1, 1), (0, 0)))
    w0 = conv_w[:, 0, 0]
    w1 = conv_w[:, 0, 1]
    w2 = conv_w[:, 0, 2]
    xc = xp[:, :-2, :] * w0 + xp[:, 1:-1, :] * w1 + xp[:, 2:, :] * w2
    xr = x + xc + conv_b

    flat = xr.reshape(B, S // KF, IN).reshape(-1, IN)

    ms = np.mean(flat * flat, axis=-1, keepdims=True, dtype=np.float32)
    n = (flat * (1.0 / np.sqrt(ms + EPS_RMS)) * rms_w).astype(np.float32)

    logits = n @ router_w.T
    probs = 1.0 / (1.0 + np.exp(-logits))
    order = np.argsort(-probs, axis=1, kind="stable")
    idx = order[:, :TOPK]
    scores = np.take_along_axis(probs, idx, axis=1)
    w = scores / (scores.sum(axis=1, keepdims=True) + EPS_W)
    combine = np.zeros((n.shape[0], E), np.float32)
    rows = np.arange(n.shape[0])
    for j in range(TOPK):
        combine[rows, idx[:, j]] = w[:, j]
    return n, combine


def build_nc(TE, cnt=None, reps=1):
    """One SPMD program for all 8 cores.

    TE: padded per-expert token count (layout size, multiple of 128).
    cnt: actual max token count over experts (compute bound, <= TE).
    reps>1 wraps the body in a hardware loop (benchmark use only: repeats
    are idempotent; used for differential wall-clock timing).
    """
    if cnt is None:
        cnt = TE
    TT = TE // P
    dt = mybir.dt
    nc = bacc.Bacc()

    resident_etok = TE <= 768   # pathological routing falls back to streaming

    ntok_d = nc.dram_tensor("ntok", [2, NKG, P, KG, 512], dt.bfloat16, kind="ExternalInput")
    ew1t_d = nc.dram_tensor("ew1t", [ME, P, KT, P], dt.bfloat16, kind="ExternalInput")
    ew2t_d = nc.dram_tensor("ew2t", [P, ME, O], dt.bfloat16, kind="ExternalInput")
    w1sh_d = nc.dram_tensor("w1sh", [P, KT, H1S], dt.bfloat16, kind="ExternalInput")
    w2sh_d = nc.dram_tensor("w2sh", [P, MS, O], dt.bfloat16, kind="ExternalInput")
    b1e_d = nc.dram_tensor("b1e", [P, ME], dt.float32, kind="ExternalInput")
    b1s_d = nc.dram_tensor("b1s", [P, MS], dt.float32, kind="ExternalInput")
    esc_d = nc.dram_tensor("esc", [P, TT], dt.float32, kind="ExternalInput")
    oute_d = nc.dram_tensor("oute", [TE, O], dt.bfloat16, kind="ExternalOutput")
    outs_d = nc.dram_tensor("outs", [TK, O], dt.bfloat16, kind="ExternalOutput")

    nch = _chunks(cnt, 512)      # expert mm1 token chunks (balanced)
    if not resident_etok:
        nch = [(0, 512), (512, cnt - 512)]
    # mm2 row tiles MUST be 128-aligned: esc is striped as esc[p, t] = scale
    # of token t*128+p, so each row tile has to start at a multiple of 128.
    tch = [(i * P, min(P, cnt - i * P)) for i in range((cnt + P - 1) // P)]
    relu = mybir.ActivationFunctionType.Relu

    with tile.TileContext(nc) as tc:
        with (
            tc.tile_pool(name="res", bufs=1) as res,
            tc.tile_pool(name="wp", bufs=2) as wp,
            tc.tile_pool(name="npl", bufs=4) as npl,
            tc.tile_pool(name="opl", bufs=3) as opl,
            tc.tile_pool(name="psp", bufs=8, space="PSUM") as psp,
        ):

            def emit_body():
                ew2t = res.tile([P, ME, O], dt.bfloat16, name="ew2t")
                w1sh = res.tile([P, KT, H1S], dt.bfloat16, name="w1sh")
                w2sh = res.tile([P, MS, O], dt.bfloat16, name="w2sh")
                b1e = res.tile([P, ME], dt.float32, name="b1e")
                b1s = res.tile([P, MS], dt.float32, name="b1s")
                esc = res.tile([P, TT], dt.float32, name="esc")
                hte = res.tile([P, ME, TE], dt.bfloat16, name="hte")
                hts = res.tile([P, MS, TK], dt.bfloat16, name="hts")

                # DMA issue order == consumption order, all on the SP queue.
                # The SP queue is FIFO and a transfer cannot issue until its
                # dest slot frees, so buffer-gated streams (nt, wt) must not
                # sit ahead of urgent loads. Order: first ntok groups (not
                # yet gated), then the expert-side prefetch (etok, first two
                # wt), then the gated remainder.
                etok_res = None
                if resident_etok:
                    etok_res = res.tile([P, NKG, KG, TE], dt.bfloat16, name="etok")

                nc.sync.dma_start(w1sh[:, 0:KG], w1sh_d[:, 0:KG])
                nc.sync.dma_start(b1s[:], b1s_d[:])

                pre_nt = []
                for g in range(min(3, NKG)):
                    nt = npl.tile([P, KG, 512], dt.bfloat16, tag="ntok", name="nt")
                    nc.sync.dma_start(nt[:], ntok_d[0, g])
                    pre_nt.append(nt)
                for g in range(1, NKG):
                    nc.sync.dma_start(
                        w1sh[:, g * KG : (g + 1) * KG], w1sh_d[:, g * KG : (g + 1) * KG]
                    )

                if NKG > 3:
                    nt4 = npl.tile([P, KG, 512], dt.bfloat16, tag="ntok", name="nt4")
                    nc.sync.dma_start(nt4[:], ntok_d[0, 3])
                    pre_nt.append(nt4)
                pre_wt = []
                for m in range(1):
                    wt = wp.tile([P, KT, P], dt.bfloat16, tag="w1e", name="wt")
                    nc.sync.dma_start(wt[:], ew1t_d[m])
                    pre_wt.append(wt)

                # ---- shared mm1: hts[:, m, :] = relu(w1sh[m].T @ ntok + b1s[m]) ----
                for ci in range(2):
                    pss = [
                        psp.tile([P, 512], dt.float32, tag="ps", name="ps_s1")
                        for _ in range(MS)
                    ]
                    for g in range(NKG):
                        if ci == 0 and g < len(pre_nt):
                            nt = pre_nt[g]
                        else:
                            nt = npl.tile([P, KG, 512], dt.bfloat16, tag="ntok", name="nt")
                            nc.sync.dma_start(nt[:], ntok_d[ci, g])
                        cw = min(512, max(0, cnt - ci * 512))
                        for kk in range(KG):
                            k = g * KG + kk
                            if resident_etok and cw > 0:
                                nc.vector.tensor_copy(
                                    etok_res[:, g, kk, ci * 512 : ci * 512 + cw],
                                    nt[:, kk, :cw],
                                )
                            for m in range(MS):
                                nc.tensor.matmul(
                                    pss[m],
                                    w1sh[:, k, m * P : (m + 1) * P],
                                    nt[:, kk],
                                    start=(k == 0),
                                    stop=(k == KT - 1),
                                )
                    for m in range(MS):
                        nc.scalar.activation(
                            hts[:, m, ci * 512 : (ci + 1) * 512],
                            pss[m],
                            relu,
                            bias=b1s[:, m : m + 1],
                            scale=1.0,
                        )

                nc.sync.dma_start(b1e[:], b1e_d[:])

                # ---- expert mm1: hte[:, m, :cnt] = relu(ew1t[m].T @ etok + b1e[m]) ----
                for m in range(ME):
                    if m < len(pre_wt):
                        wt = pre_wt[m]
                    else:
                        wt = wp.tile([P, KT, P], dt.bfloat16, tag="w1e", name="wt")
                        nc.sync.dma_start(wt[:], ew1t_d[m])
                    pss = [
                        psp.tile([P, 512], dt.float32, tag="ps", name="ps_e1")[:, :w]
                        for (_, w) in nch
                    ]
                    if resident_etok:
                        for k in range(KT):
                            for ci, (off, w) in enumerate(nch):
                                nc.tensor.matmul(
                                    pss[ci],
                                    wt[:, k],
                                    etok_res[:, k // KG, k % KG, off : off + w],
                                    start=(k == 0),
                                    stop=(k == KT - 1),
                                )
                    else:
                        # pathological routing (>768 tokens on one expert):
                        # re-stream 512-aligned token windows; slow but correct
                        for g in range(NKG):
                            nt2s = []
                            for ci, (off, w) in enumerate(nch):
                                nt2 = npl.tile(
                                    [P, KG, 512], dt.bfloat16, tag="ntok", name="nt2"
                                )
                                nc.sync.dma_start(nt2[:, :, :w], ntok_d[ci, g][:, :, :w])
                                nt2s.append(nt2)
                            for kk in range(KG):
                                k = g * KG + kk
                                for ci, (off, w) in enumerate(nch):
                                    nc.tensor.matmul(
                                        pss[ci],
                                        wt[:, k],
                                        nt2s[ci][:, kk, :w],
                                        start=(k == 0),
                                        stop=(k == KT - 1),
                                    )
                    for ci, (off, w) in enumerate(nch):
                        nc.scalar.activation(
                            hte[:, m, off : off + w],
                            pss[ci],
                            relu,
                            bias=b1e[:, m : m + 1],
                            scale=1.0,
                        )

                nc.sync.dma_start(w2sh[:], w2sh_d[:])
                nc.sync.dma_start(esc[:], esc_d[:])
                nc.sync.dma_start(ew2t[:], ew2t_d[:])

                # ---- shared mm2: outs rows = hts.T @ w2sh ----
                for t in range(TK // P):
                    pso = [
                        psp.tile([P, 512], dt.float32, tag="ps", name="ps_o")
                        for _ in range(NO)
                    ]
                    for k in range(MS):
                        for o in range(NO):
                            nc.tensor.matmul(
                                pso[o],
                                hts[:, k, t * P : (t + 1) * P],
                                w2sh[:, k, o * 512 : (o + 1) * 512],
                                start=(k == 0),
                                stop=(k == MS - 1),
                            )
                    ot = opl.tile([P, O], dt.bfloat16, tag="out", name="ot_s")
                    for o in range(NO):
                        nc.vector.tensor_copy(ot[:, o * 512 : (o + 1) * 512], pso[o])
                    nc.sync.dma_start(outs_d[t * P : (t + 1) * P], ot[:])

                # ---- expert mm2: oute rows = (hte.T @ ew2t) * esc ----
                for t, (toff, tw) in enumerate(tch):
                    pso = [
                        psp.tile([P, 512], dt.float32, tag="ps", name="ps_o")
                        for _ in range(NO)
                    ]
                    for k in range(ME):
                        for o in range(NO):
                            nc.tensor.matmul(
                                pso[o][:tw],
                                hte[:, k, toff : toff + tw],
                                ew2t[:, k, o * 512 : (o + 1) * 512],
                                start=(k == 0),
                                stop=(k == ME - 1),
                            )
                    ot = opl.tile([P, O], dt.bfloat16, tag="out", name="ot_e")
                    for o in range(NO):
                        nc.vector.tensor_scalar_mul(
                            ot[:tw, o * 512 : (o + 1) * 512],
                            pso[o][:tw],
                            esc[:tw, t : t + 1],
                        )
                    nc.sync.dma_start(oute_d[toff : toff + tw], ot[:tw])

            if reps == 1:
                emit_body()
            else:
                with tc.For_i(0, reps, 1):
                    emit_body()

    nc.finalize()
    return nc


def _prepare(inputs):
    inp = {k: np.asarray(v, dtype=np.float32) for k, v in inputs.items()}
    n, combine = host_preprocess(
        inp["x"], inp["conv_w"], inp["conv_b"], inp["rms_w"], inp["router_w"]
    )
    nbf = n.astype(BF16)

    idxs = [np.nonzero(combine[:, e] > 0)[0] for e in range(E)]
    maxcnt = max(1, max(len(ix) for ix in idxs))
    TE = int(math.ceil(maxcnt / P) * P)
    TT = TE // P

    all_tokens = np.arange(TK)
    perms = []
    in_maps = []
    for c in range(NCORES):
        e, hh = divmod(c, 2)
        sl = slice(hh * H1E, (hh + 1) * H1E)
        # ew1t[m, p, k, q] = W1h[m*128+q, k*128+p]  (lhsT layout, contiguous per (m,p))
        W1h = inp["ew1"][e, sl]                      # [H1E, IN]
        ew1t = np.ascontiguousarray(
            W1h.reshape(ME, P, KT, P).transpose(0, 3, 2, 1)
        ).astype(BF16)
        W2h = inp["ew2"][e][:, sl]                   # [O, H1E]
        ew2t = np.ascontiguousarray(
            W2h.T.reshape(ME, P, O).transpose(1, 0, 2)
        ).astype(BF16)
        ssl = slice(c * H1S, (c + 1) * H1S)
        w1sh = np.ascontiguousarray(
            inp["sw1"][ssl].T.reshape(KT, P, H1S).transpose(1, 0, 2)
        ).astype(BF16)
        w2sh = np.ascontiguousarray(
            inp["sw2"][:, ssl].T.reshape(MS, P, O).transpose(1, 0, 2)
        ).astype(BF16)
        b1e = np.ascontiguousarray(inp["eb1"][e, sl].reshape(ME, P).T).astype(np.float32)
        b1s = np.ascontiguousarray(inp["sb1"][ssl].reshape(MS, P).T).astype(np.float32)

        idx_e = idxs[e]
        cnt = len(idx_e)
        # permute tokens so this core's expert tokens come first; the expert
        # matmuls then reuse the prefix of the shared-expert token stream
        mask = np.zeros(TK, bool)
        mask[idx_e] = True
        perm = np.concatenate([idx_e, all_tokens[~mask]])
        perms.append(perm)
        ntok = np.ascontiguousarray(
            nbf[perm].T.reshape(NKG, KG, P, 2, 512).transpose(3, 0, 2, 1, 4)
        )
        esc = np.zeros((TE,), np.float32)
        esc[:cnt] = combine[idx_e, e]
        escp = np.ascontiguousarray(esc.reshape(TT, P).T)

        in_maps.append(
            {
                "ntok": ntok,
                "ew1t": ew1t,
                "ew2t": ew2t,
                "w1sh": w1sh,
                "w2sh": w2sh,
                "b1e": b1e,
                "b1s": b1s,
                "esc": escp,
            }
        )
    return inp, combine, idxs, perms, TE, in_maps


def _assemble(inp, combine, idxs, perms, results):
    acc = np.zeros((TK, O), np.float32)
    for c in range(NCORES):
        acc[perms[c]] += results[c]["outs"].astype(np.float32)
    acc += inp["sb2"][None, :]
    acc += combine @ inp["eb2"]
    for c in range(NCORES):
        e = c // 2
        idx_e = idxs[e]
        cnt = len(idx_e)
        if cnt:
            acc[idx_e] += results[c]["oute"][:cnt].astype(np.float32)
    return acc.reshape(B, S // KF, O)


def run(inputs, trace=False):
    inp, combine, idxs, perms, TE, in_maps = _prepare(inputs)
    maxcnt = max(1, max(len(ix) for ix in idxs))
    nc = build_nc(TE, cnt=maxcnt)
    res = run_bass_kernel_spmd(nc, in_maps, core_ids=list(range(NCORES)), trace=trace)
    out = _assemble(inp, combine, idxs, perms, res.results)
    return out, res


def kernel(**inputs):
    out, _ = run(inputs, trace=False)
    return out

